# revision 36
# baseline (speedup 1.0000x reference)
"""Trainium2 Bass kernel: MultiHeadSelfAttention (B=1, S=4096, D=512, H=8, DK=DV=64)
with fc_out applied twice.

Sharding: sequence-sharded across 8 cores (512 queries per core). Every core
receives the FULL keys/values (pre-transposed, bf16) and redundantly computes
the full K^T / V projections on-device (an AllGather measured ~125us wall for
1MB on this fabric - dead); attention + the two output projections run on the
core's own 512-query chunk. Host concatenates the 8 output chunks.

Layout notes:
  - heads are processed in PAIRS, lockstep over j-tiles. The scores^T tiles
    [seq_k(128) x seq_q(512)] for BOTH heads of a pair come out of PE in one
    ~220ns window via two concurrent row-tiled K=64 matmuls (tile_position
    (0,0)/(64,0)): head 2p streams its q through lanes 0-63 while head 2p+1
    streams through lanes 64-127. This fills the rhs xbus completely - 2x the
    throughput of the old zero-padded K=128 formulation.
  - KTp packs head pairs (head 2p rows 0-63, 2p+1 rows 64-127); qTz puts even
    heads on rows 0-63 and odd heads on rows 64-127 to match.
  - softmax denominator via a ones-column appended to each head's V (stride
    65): attn@V gives [65, 512] per head = output^T rows + exp-sum row. attn@V
    streams pt through all 128 lanes already (K=128) - irreducible, unchanged.
  - raw K^T and V^T stay RESIDENT in SBUF (4MB each; no reload churn), so
    projection drips can run any time: V + K0 + K1's head drip inside pair 0
    (V tail j>=DEFER deferred into pair 1's chunk hooks via parked pt tiles),
    K pair p's remaining column-groups JIT inside pair p itself.
  - PSUM: score chunks [128,1024] x2 bufs (4 banks) + av x2 + kproj x2 = 8.
  - input DMAs are deadline-ordered (the rail moves ~10MB while pair 0 runs);
    Wo/bo issue from the gpsimd queue mid-attention so they don't steal early
    bandwidth. Output y is bf16 (host casts back to f32).
  - the chip power-caps: >~3.4us PE idle halves the clock (HAM), but packing
    the pipeline perfectly trips the P0 power state (-20%% on ALL clocks).
    The pinned keep-warm matmuls before fc bridge the finalize dependency gap
    just enough to keep HAM at 8/8 without crossing the power budget.
"""
import sys, functools
sys.path.insert(0, "/opt/trn_rl_repo")
if "/root/.axon_site" not in sys.path:
    sys.path.insert(0, "/root/.axon_site")
import numpy as np
import ml_dtypes

import concourse.bass as bass
import concourse.tile as tile
from concourse import bacc, mybir, masks
from concourse.bass_utils import run_bass_kernel_spmd

NCORES = 8
S, D, H, DK = 4096, 512, 8, 64
CH = S // NCORES            # 512 sequence rows per core
VW = H * (DK + 1)           # 520: v row width incl. ones columns
JT = S // 128               # 32 seq_k tiles
DEFER = 24                  # pair-0 j-tiles >= DEFER run inside pair 1

F32 = mybir.dt.float32
BF16 = mybir.dt.bfloat16
EXP = mybir.ActivationFunctionType.Exp


def _build_program():
    nc = bacc.Bacc("TRN2", target_bir_lowering=False, debug=False,
                   num_devices=NCORES)

    xqT = nc.dram_tensor("xqT", [D, CH], BF16, kind="ExternalInput")
    keysT = nc.dram_tensor("keysT", [D, S], BF16, kind="ExternalInput")
    valsT = nc.dram_tensor("valsT", [D, S], BF16, kind="ExternalInput")
    Wq = nc.dram_tensor("Wq", [D, D], BF16, kind="ExternalInput")
    Wk = nc.dram_tensor("Wk", [D, D], BF16, kind="ExternalInput")
    Wv = nc.dram_tensor("Wv", [D, D], BF16, kind="ExternalInput")
    Wo = nc.dram_tensor("Wo", [D, D], BF16, kind="ExternalInput")
    bo = nc.dram_tensor("bo", [D], F32, kind="ExternalInput")
    y = nc.dram_tensor("y", [CH, D], BF16, kind="ExternalOutput")

    with tile.TileContext(nc) as tc:
        with tc.tile_pool(name="persist", bufs=1) as pp, \
             tc.tile_pool(name="kv", bufs=1) as kvp:

            Wo_sb = pp.tile([128, 2048], BF16, tag="wo")
            Wk_sb = pp.tile([128, 2048], BF16, tag="wk")
            Wv_sb = pp.tile([128, 2048], BF16, tag="wv")
            bo_sb = pp.tile([128, 4], F32, tag="bo")
            ident = pp.tile([128, 128], BF16, tag="id")
            qTz_sb = pp.tile([128, H * 512], BF16, tag="qt")
            attT_sb = pp.tile([128, 2048], BF16, tag="att")
            KTp = [kvp.tile([128, S], BF16, tag=f"kt{p}", name=f"KT{p}")
                   for p in range(H // 2)]
            # V natural [seq, head-stripes of 65 (64 + ones col)]
            V_sb = kvp.tile([128, JT * VW], BF16, tag="v")

            masks.make_identity(nc, ident[:])
            nc.gpsimd.memset(
                V_sb[:].rearrange("p (j h x) -> p j h x", j=JT, h=H, x=DK + 1)
                [:, :, :, DK:DK + 1], 1.0)

            with tc.tile_pool(name="stage", bufs=1) as stp, \
                 tc.tile_pool(name="pt", bufs=13) as ptp, \
                 tc.tile_pool(name="rc", bufs=2) as rcp, \
                 tc.tile_pool(name="ps_sc", bufs=2, space="PSUM") as pssc, \
                 tc.tile_pool(name="ps_kp", bufs=2, space="PSUM") as pskp, \
                 tc.tile_pool(name="ps_av", bufs=1, space="PSUM") as psav:

                def staged_load(dst_sb, src_dram, nchunks=8, chunks=None):
                    w = S // nchunks
                    dst = dst_sb[:].rearrange("p (k s) -> p k s", k=4)
                    srcv = src_dram.ap().rearrange("(k p) s -> p k s", p=128)
                    for ci in (range(nchunks) if chunks is None else chunks):
                        nc.sync.dma_start(dst[:, :, w * ci:w * ci + w],
                                          srcv[:, :, w * ci:w * ci + w])

                def q_proj():
                    for m in range(4):
                        ps = pskp.tile([128, 512], F32, tag="kp", name=f"qp{m}")
                        for k in range(4):
                            nc.tensor.matmul(
                                ps[:], lhsT=Wq_sb[:, 512 * k + 128 * m:512 * k + 128 * m + 128],
                                rhs=xqT_sb[:, 512 * k:512 * k + 512],
                                start=(k == 0), stop=(k == 3))
                        nc.vector.tensor_copy(
                            qTz_sb[0:64, 512 * (2 * m):512 * (2 * m) + 512], ps[0:64, :])
                        nc.vector.tensor_copy(
                            qTz_sb[64:128, 512 * (2 * m + 1):512 * (2 * m + 1) + 512],
                            ps[64:128, :])

                def v_proj_group(j):
                    ps = pskp.tile([128, 512], F32, tag="kp", name=f"vp{j}")
                    for k in range(4):
                        nc.tensor.matmul(
                            ps[:], lhsT=vst[:, S * k + 128 * j:S * k + 128 * j + 128],
                            rhs=Wv_sb[:, 512 * k:512 * k + 512],
                            start=(k == 0), stop=(k == 3))
                    dst = V_sb[:, VW * j:VW * j + VW].rearrange(
                        "p (h x) -> p h x", h=H, x=DK + 1)[:, :, 0:DK]
                    nc.vector.tensor_copy(
                        dst, ps[:].rearrange("p (h x) -> p h x", h=H, x=DK))

                def k_proj_group(m, sc):
                    ps = pskp.tile([128, 512], F32, tag="kp", name=f"kp{m}_{sc}")
                    for k in range(4):
                        nc.tensor.matmul(
                            ps[:], lhsT=Wk_sb[:, 512 * k + 128 * m:512 * k + 128 * m + 128],
                            rhs=kst[:, S * k + 512 * sc:S * k + 512 * sc + 512],
                            start=(k == 0), stop=(k == 3))
                    nc.vector.tensor_copy(KTp[m][:, 512 * sc:512 * sc + 512], ps[:])

                def attn_v_unit(p, j, pt, av1, av2):
                    # both heads of pair p against j's V stripes; pt holds
                    # [h1 | h2] score-exp columns for this j
                    h1, h2 = 2 * p, 2 * p + 1
                    nc.tensor.matmul(
                        av1[:], lhsT=V_sb[:, VW * j + 65 * h1:VW * j + 65 * h1 + 65],
                        rhs=pt[:, 0:512],
                        start=(j == 0), stop=(j == JT - 1))
                    nc.tensor.matmul(
                        av2[:], lhsT=V_sb[:, VW * j + 65 * h2:VW * j + 65 * h2 + 65],
                        rhs=pt[:, 512:1024],
                        start=(j == 0), stop=(j == JT - 1))

                def finalize_head(h, av):
                    hp, hl = h // 2, h % 2
                    avc = rcp.tile([65, 512], F32, tag="avc", name=f"avc{h}")
                    rtmp = rcp.tile([1, 512], F32, tag="rt", name=f"rt{h}")
                    # denominator chain first - it's the critical path
                    # (broadcast + recip) - the big copy overlaps it
                    nc.vector.tensor_copy(rtmp[:], av[64:65, :])
                    rbc = rcp.tile([64, 512], F32, tag="rb", name=f"rb{h}")
                    rb2 = rcp.tile([64, 512], F32, tag="rb2", name=f"rb2{h}")
                    nc.gpsimd.partition_broadcast(rbc[:], rtmp[:])
                    nc.vector.tensor_copy(avc[:], av[:])
                    nc.vector.reciprocal_approx_fast(out=rb2[:], in_=rbc[:])
                    nc.vector.tensor_mul(
                        attT_sb[64 * hl:64 * hl + 64, 512 * hp:512 * hp + 512],
                        avc[0:64, :], rb2[:])

                def attention_pair(p, hooks, hold=0, lag=2):
                    """hooks: dict chunk->list of thunks run after that chunk's exp.
                    hold: delay attn@V issue until this chunk (issue-order guard
                    when the previous pair's deferred tail runs in our hooks).
                    lag: chunks between a unit's exp and its attn@V (deeper in
                    pair 0 so the V-proj drips ride the vst DMA curve)."""
                    h1, h2 = 2 * p, 2 * p + 1
                    av1 = psav.tile([65, 512], F32, tag="av1", name=f"av{h1}")
                    av2 = psav.tile([65, 512], F32, tag="av2", name=f"av{h2}")
                    q1 = qTz_sb[0:64, 512 * h1:512 * h1 + 512]
                    q2 = qTz_sb[64:128, 512 * h2:512 * h2 + 512]
                    backlog = []
                    pts = {}
                    for j in range(JT):
                        ps = pssc.tile([128, 1024], F32, tag="sc", name=f"sc{p}_{j}")
                        pt = ptp.tile([128, 1024], BF16, tag="pt", name=f"pt{p}_{j}")
                        nc.tensor.matmul(
                            ps[:, 0:512], lhsT=KTp[p][0:64, 128 * j:128 * j + 128],
                            rhs=q1, start=True, stop=True, tile_position=(0, 0))
                        nc.tensor.matmul(
                            ps[:, 512:1024], lhsT=KTp[p][64:128, 128 * j:128 * j + 128],
                            rhs=q2, start=True, stop=True, tile_position=(64, 0))
                        nc.scalar.activation(pt[:], ps[:], EXP, scale=0.125)
                        pts[j] = pt
                        for fn in hooks.get(j, ()):
                            fn()
                        if j >= lag and not (p == 0 and j - lag >= DEFER):
                            backlog.append(j - lag)
                        if j >= hold:
                            for bj in backlog:
                                attn_v_unit(p, bj, pts[bj], av1, av2)
                            backlog.clear()
                    for bj in backlog:
                        attn_v_unit(p, bj, pts[bj], av1, av2)
                    for bj in range(JT - lag, JT):
                        if not (p == 0 and bj >= DEFER):
                            attn_v_unit(p, bj, pts[bj], av1, av2)
                    return av1, av2, pts

                # ---- prolog: stage everything; q proj; K0 sc0 ----
                Wq_sb = stp.tile([128, 2048], BF16, tag="wq")
                xqT_sb = stp.tile([128, 2048], BF16, tag="xq")
                kst = stp.tile([128, 4 * S], BF16, tag="ks", name="kstage")
                vst = stp.tile([128, 4 * S], BF16, tag="vs", name="vstage")
                # per-k-chunk loads let the first q_proj matmuls start as
                # soon as ~256KB has landed instead of waiting for 1MB
                for k in range(4):
                    nc.sync.dma_start(
                        Wq_sb[:].rearrange("p (k n) -> p k n", k=4)[:, k, :],
                        Wq.ap().rearrange("(k p) n -> p k n", p=128)[:, k, :])
                    nc.sync.dma_start(
                        xqT_sb[:].rearrange("p (k s) -> p k s", k=4)[:, k, :],
                        xqT.ap().rearrange("(k p) s -> p k s", p=128)[:, k, :])
                nc.sync.dma_start(
                    Wk_sb[:].rearrange("p (k n) -> p k n", k=4),
                    Wk.ap().rearrange("(k p) n -> p k n", p=128))
                # interleave K / V staging by consumption deadline (the DMA
                # rail is the prolog+pair-0 limiter: ~10MB must land while
                # pair 0 runs). kst's first two 512-col chunks lead so the
                # first score chunks start ~8us earlier.
                staged_load(kst, keysT, nchunks=8, chunks=[0, 1])
                nc.sync.dma_start(
                    Wv_sb[:].rearrange("p (k n) -> p k n", k=4),
                    Wv.ap().rearrange("(k p) n -> p k n", p=128))
                staged_load(vst, valsT, nchunks=4, chunks=[0])
                for ci in range(1, 4):
                    staged_load(kst, keysT, nchunks=4, chunks=[ci])
                    staged_load(vst, valsT, nchunks=4, chunks=[ci])
                # Wo/bo aren't needed until fc (~200us): issue their DMAs from
                # the DVE queue after pair-1's copies so they don't steal
                # bandwidth from the attention-critical loads above
                def late_loads():
                    nc.gpsimd.dma_start(
                        Wo_sb[:].rearrange("p (k n) -> p k n", k=4),
                        Wo.ap().rearrange("(k p) n -> p k n", p=128))
                    nc.gpsimd.dma_start(bo_sb[:],
                                        bo.ap().rearrange("(m p) -> p m", p=128))

                q_proj()
                k_proj_group(0, 0)
                v_proj_group(0)
                v_proj_group(1)

                # ---- pair 0: V proj + K0 dripped JIT; K1 in the tail
                #      (pair 0 is DMA-stretched, so it has the PE slack) ----
                hooks0 = {}
                for s in range(1, 8):           # K0 sc s by chunk 4s
                    hooks0.setdefault(4 * s - 2, []).append(
                        lambda s=s: k_proj_group(0, s))
                for j in range(2, DEFER):       # V[j] by chunk j+1 (attnV lag 2)
                    hooks0.setdefault(j, []).append(
                        lambda j=j: v_proj_group(j))
                for s in range(4):              # K1 sc0-3 in pair-0's tail
                    hooks0.setdefault(26 + s, []).append(
                        lambda s=s: k_proj_group(1, s))
                av1_0, av2_0, pts0 = attention_pair(0, hooks0)

                # ---- pairs 1-3; pair-0 tail (V proj + attnV + finalize)
                #      runs inside pair 1's first hooks ----
                def make_hooks(p):
                    hooks = {}
                    if p == 1:
                        def tail(j):
                            def fn():
                                v_proj_group(j)
                                attn_v_unit(0, j, pts0[j], av1_0, av2_0)
                                if j == JT - 1:
                                    finalize_head(0, av1_0)
                                    finalize_head(1, av2_0)
                                    late_loads()
                            return fn
                        spread = [0, 1, 2, 4, 5, 6, 8, 9]
                        for i, j in enumerate(range(DEFER, JT)):
                            hooks.setdefault(spread[i], []).append(tail(j))
                    # own K tail JIT (sc s needed by chunk 4s); next pair's
                    # leading column-groups prime in our later hooks
                    if p == 1:
                        for s in range(4, 8):
                            hooks.setdefault(4 * s - 3, []).append(
                                lambda s=s: k_proj_group(p, s))
                        for i in range(2):
                            hooks.setdefault(26 + i, []).append(
                                lambda s=i: k_proj_group(2, s))
                    else:
                        for s in range(2, 8):
                            hooks.setdefault(4 * s - 3, []).append(
                                lambda s=s: k_proj_group(p, s))
                        if p < 3:
                            for i in range(2):
                                hooks.setdefault(26 + i, []).append(
                                    lambda s=i: k_proj_group(p + 1, s))
                    return hooks

                pts3 = None
                for p in (1, 2, 3):
                    av1, av2, ptsp = attention_pair(p, make_hooks(p),
                                                   hold=(11 if p == 1 else 0))
                    finalize_head(2 * p, av1)
                    finalize_head(2 * p + 1, av2)
                    pts3 = ptsp

                # keep PE busy across the finalize->fc dependency gap (~5us):
                # a >3.4us PE idle here re-throttles the HAM clock to 1.2GHz
                # and the whole fc phase then runs at half rate
                # rhs reads pair-3's late pt tiles so the static scheduler
                # cannot hoist these ahead of the tail of the attention phase
                warmf = pskp.tile([128, 512], F32, tag="kp", name="warmf")
                for i in range(18):
                    src = pts3[28 + (i % 4)]
                    nc.tensor.matmul(warmf[:], lhsT=ident[:],
                                     rhs=src[:, 0:512],
                                     start=True, stop=True)

            # ---- fc_out twice (fp32), then transpose to natural layout ----
            with tc.tile_pool(name="fc", bufs=3) as fcp, \
                 tc.tile_pool(name="ps_fc", bufs=2, space="PSUM") as psfc, \
                 tc.tile_pool(name="ps_tr", bufs=2, space="PSUM") as pstr:
                o1T = fcp.tile([128, 2048], BF16, tag="fcb", name="o1T")
                o2T = fcp.tile([128, 2048], BF16, tag="fcb", name="o2T")
                onat = fcp.tile([128, 2048], BF16, tag="fcb", name="onat")
                yv = y.ap().rearrange("(m p) f -> p m f", m=4, p=128)

                def fc_block(fsrc, fdst, m):
                    ps = psfc.tile([128, 512], F32, tag="fc")
                    for k in range(4):
                        nc.tensor.matmul(
                            ps[:], lhsT=Wo_sb[:, 512 * k + 128 * m:512 * k + 128 * m + 128],
                            rhs=fsrc[:, 512 * k:512 * k + 512],
                            start=(k == 0), stop=(k == 3))
                    nc.vector.tensor_scalar_add(
                        fdst[:, 512 * m:512 * m + 512], ps[:], bo_sb[:, m:m + 1])

                for m in range(4):
                    fc_block(attT_sb, o1T, m)
                for m in range(4):
                    fc_block(o1T, o2T, m)
                    # transposes of fc2 block m can start right away
                    # (kf-major): they hide under the next fc_block
                    for mm in range(4):
                        pst = pstr.tile([128, 128], BF16, tag="tr")
                        nc.tensor.transpose(
                            pst[:], o2T[:, 512 * m + 128 * mm:512 * m + 128 * mm + 128],
                            ident[:])
                        nc.vector.tensor_copy(
                            onat[:, 512 * mm + 128 * m:512 * mm + 128 * m + 128],
                            pst[:])
                    if m == 3:
                        for mm in range(4):
                            nc.sync.dma_start(
                                yv[:, mm, :],
                                onat[:, 512 * mm:512 * mm + 512])

    nc.compile()
    return nc


@functools.lru_cache(maxsize=1)
def _get_program():
    return _build_program()


def _make_in_maps(queries, keys, values, Wq, Wk, Wv, Wo, bo):
    q = np.asarray(queries, np.float32).reshape(S, D)
    kT = np.ascontiguousarray(np.asarray(keys, np.float32).reshape(S, D).T
                              ).astype(ml_dtypes.bfloat16)
    vT = np.ascontiguousarray(np.asarray(values, np.float32).reshape(S, D).T
                              ).astype(ml_dtypes.bfloat16)
    Wq = np.ascontiguousarray(np.asarray(Wq, np.float32)).astype(ml_dtypes.bfloat16)
    Wk = np.ascontiguousarray(np.asarray(Wk, np.float32)).astype(ml_dtypes.bfloat16)
    Wv = np.ascontiguousarray(np.asarray(Wv, np.float32)).astype(ml_dtypes.bfloat16)
    Wo = np.ascontiguousarray(np.asarray(Wo, np.float32)).astype(ml_dtypes.bfloat16)
    bo = np.ascontiguousarray(np.asarray(bo, np.float32))
    in_maps = []
    for c in range(NCORES):
        in_maps.append({
            "xqT": np.ascontiguousarray(q[c * CH:(c + 1) * CH].T).astype(ml_dtypes.bfloat16),
            "keysT": kT, "valsT": vT,
            "Wq": Wq, "Wk": Wk, "Wv": Wv, "Wo": Wo, "bo": bo,
        })
    return in_maps


def _run(in_maps, **kw):
    nc = _get_program()
    return run_bass_kernel_spmd(nc, in_maps, core_ids=list(range(NCORES)), **kw)


def kernel(queries, keys, values, Wq, Wk, Wv, Wo, bo):
    res = _run(_make_in_maps(queries, keys, values, Wq, Wk, Wv, Wo, bo))
    out = np.concatenate([res.results[c]["y"] for c in range(NCORES)],
                         axis=0).astype(np.float32)
    return out.reshape(1, S, D)


def run_traced(queries, keys, values, Wq, Wk, Wv, Wo, bo):
    """Like kernel() but with NTFF profiling; returns (output, BassKernelResults)."""
    import types
    import trn_agent_boot.trn_boot as _tb
    from concourse import bass_utils
    hook = _tb._ntff_profile_via_ctypes("/opt/axon/libaxon_pjrt.so")
    mod = types.ModuleType("antenv.axon_hooks")
    mod.get_axon_ntff_profile_hook = lambda: hook
    sys.modules["antenv.axon_hooks"] = mod
    bass_utils.upload_artifacts = lambda tmpdir: tmpdir
    res = _run(_make_in_maps(queries, keys, values, Wq, Wk, Wv, Wo, bo), trace=True)
    out = np.concatenate([res.results[c]["y"] for c in range(NCORES)],
                         axis=0).astype(np.float32)
    return out.reshape(1, S, D), res


# revision 38
# speedup vs baseline: 1.0021x; 1.0021x over previous
"""Trainium2 Bass kernel: MultiHeadSelfAttention (B=1, S=4096, D=512, H=8, DK=DV=64)
with fc_out applied twice.

Sharding: sequence-sharded across 8 cores (512 queries per core). Every core
receives the FULL keys/values (pre-transposed, bf16) and redundantly computes
the full K^T / V projections on-device (an AllGather measured ~125us wall for
1MB on this fabric - dead); attention + the two output projections run on the
core's own 512-query chunk. Host concatenates the 8 output chunks.

Layout notes:
  - heads are processed in PAIRS, lockstep over j-tiles. The scores^T tiles
    [seq_k(128) x seq_q(512)] for BOTH heads of a pair come out of PE in one
    ~220ns window via two concurrent row-tiled K=64 matmuls (tile_position
    (0,0)/(64,0)): head 2p streams its q through lanes 0-63 while head 2p+1
    streams through lanes 64-127. This fills the rhs xbus completely - 2x the
    throughput of the old zero-padded K=128 formulation.
  - KTp packs head pairs (head 2p rows 0-63, 2p+1 rows 64-127); qTz puts even
    heads on rows 0-63 and odd heads on rows 64-127 to match.
  - softmax denominator via a ones-column appended to each head's V (stride
    65): attn@V gives [65, 512] per head = output^T rows + exp-sum row. attn@V
    streams pt through all 128 lanes already (K=128) - irreducible, unchanged.
  - raw K^T and V^T stay RESIDENT in SBUF (4MB each; no reload churn), so
    projection drips can run any time: V + K0 + K1's head drip inside pair 0
    (V tail j>=DEFER deferred into pair 1's chunk hooks via parked pt tiles),
    K pair p's remaining column-groups JIT inside pair p itself.
  - PSUM: score chunks [128,1024] x2 bufs (4 banks) + av x2 + kproj x2 = 8.
  - input DMAs are deadline-ordered (the rail moves ~10MB while pair 0 runs);
    Wo/bo issue from the gpsimd queue mid-attention so they don't steal early
    bandwidth. Output y is bf16 (host casts back to f32).
  - the chip power-caps: >~3.4us PE idle halves the clock (HAM), but packing
    the pipeline perfectly trips the P0 power state (-20%% on ALL clocks).
    The pinned keep-warm matmuls before fc bridge the finalize dependency gap
    just enough to keep HAM at 8/8 without crossing the power budget.
"""
import sys, functools
sys.path.insert(0, "/opt/trn_rl_repo")
if "/root/.axon_site" not in sys.path:
    sys.path.insert(0, "/root/.axon_site")
import numpy as np
import ml_dtypes

import concourse.bass as bass
import concourse.tile as tile
from concourse import bacc, mybir, masks
from concourse.bass_utils import run_bass_kernel_spmd

NCORES = 8
S, D, H, DK = 4096, 512, 8, 64
CH = S // NCORES            # 512 sequence rows per core
VW = H * (DK + 1)           # 520: v row width incl. ones columns
JT = S // 128               # 32 seq_k tiles
DEFER = 24                  # pair-0 j-tiles >= DEFER run inside pair 1

F32 = mybir.dt.float32
BF16 = mybir.dt.bfloat16
EXP = mybir.ActivationFunctionType.Exp


def _build_program():
    nc = bacc.Bacc("TRN2", target_bir_lowering=False, debug=False,
                   num_devices=NCORES)

    xqT = nc.dram_tensor("xqT", [D, CH], BF16, kind="ExternalInput")
    keysT = nc.dram_tensor("keysT", [D, S], BF16, kind="ExternalInput")
    valsT = nc.dram_tensor("valsT", [D, S], BF16, kind="ExternalInput")
    Wq = nc.dram_tensor("Wq", [D, D], BF16, kind="ExternalInput")
    Wk = nc.dram_tensor("Wk", [D, D], BF16, kind="ExternalInput")
    Wv = nc.dram_tensor("Wv", [D, D], BF16, kind="ExternalInput")
    Wo = nc.dram_tensor("Wo", [D, D], BF16, kind="ExternalInput")
    bo = nc.dram_tensor("bo", [D], F32, kind="ExternalInput")
    y = nc.dram_tensor("y", [CH, D], BF16, kind="ExternalOutput")

    with tile.TileContext(nc) as tc:
        with tc.tile_pool(name="persist", bufs=1) as pp, \
             tc.tile_pool(name="kv", bufs=1) as kvp:

            Wo_sb = pp.tile([128, 2048], BF16, tag="wo")
            Wk_sb = pp.tile([128, 2048], BF16, tag="wk")
            Wv_sb = pp.tile([128, 2048], BF16, tag="wv")
            bo_sb = pp.tile([128, 4], F32, tag="bo")
            ident = pp.tile([128, 128], BF16, tag="id")
            qTz_sb = pp.tile([128, H * 512], BF16, tag="qt")
            attT_sb = pp.tile([128, 2048], BF16, tag="att")
            KTp = [kvp.tile([128, S], BF16, tag=f"kt{p}", name=f"KT{p}")
                   for p in range(H // 2)]
            # V natural [seq, head-stripes of 65 (64 + ones col)]
            V_sb = kvp.tile([128, JT * VW], BF16, tag="v")

            masks.make_identity(nc, ident[:])
            nc.gpsimd.memset(
                V_sb[:].rearrange("p (j h x) -> p j h x", j=JT, h=H, x=DK + 1)
                [:, :, :, DK:DK + 1], 1.0)

            with tc.tile_pool(name="stage", bufs=1) as stp, \
                 tc.tile_pool(name="pt", bufs=13) as ptp, \
                 tc.tile_pool(name="rc", bufs=2) as rcp, \
                 tc.tile_pool(name="ps_sc", bufs=2, space="PSUM") as pssc, \
                 tc.tile_pool(name="ps_kp", bufs=2, space="PSUM") as pskp, \
                 tc.tile_pool(name="ps_av", bufs=1, space="PSUM") as psav:

                def staged_load(dst_sb, src_dram, nchunks=8, chunks=None):
                    w = S // nchunks
                    dst = dst_sb[:].rearrange("p (k s) -> p k s", k=4)
                    srcv = src_dram.ap().rearrange("(k p) s -> p k s", p=128)
                    for ci in (range(nchunks) if chunks is None else chunks):
                        nc.sync.dma_start(dst[:, :, w * ci:w * ci + w],
                                          srcv[:, :, w * ci:w * ci + w])

                def q_proj():
                    for m in range(4):
                        ps = pskp.tile([128, 512], F32, tag="kp", name=f"qp{m}")
                        for k in range(4):
                            nc.tensor.matmul(
                                ps[:], lhsT=Wq_sb[:, 512 * k + 128 * m:512 * k + 128 * m + 128],
                                rhs=xqT_sb[:, 512 * k:512 * k + 512],
                                start=(k == 0), stop=(k == 3))
                        nc.vector.tensor_copy(
                            qTz_sb[0:64, 512 * (2 * m):512 * (2 * m) + 512], ps[0:64, :])
                        nc.vector.tensor_copy(
                            qTz_sb[64:128, 512 * (2 * m + 1):512 * (2 * m + 1) + 512],
                            ps[64:128, :])

                def v_proj_group(j):
                    ps = pskp.tile([128, 512], F32, tag="kp", name=f"vp{j}")
                    for k in range(4):
                        nc.tensor.matmul(
                            ps[:], lhsT=vst[:, S * k + 128 * j:S * k + 128 * j + 128],
                            rhs=Wv_sb[:, 512 * k:512 * k + 512],
                            start=(k == 0), stop=(k == 3))
                    dst = V_sb[:, VW * j:VW * j + VW].rearrange(
                        "p (h x) -> p h x", h=H, x=DK + 1)[:, :, 0:DK]
                    nc.vector.tensor_copy(
                        dst, ps[:].rearrange("p (h x) -> p h x", h=H, x=DK))

                def k_proj_group(m, sc):
                    ps = pskp.tile([128, 512], F32, tag="kp", name=f"kp{m}_{sc}")
                    for k in range(4):
                        nc.tensor.matmul(
                            ps[:], lhsT=Wk_sb[:, 512 * k + 128 * m:512 * k + 128 * m + 128],
                            rhs=kst[:, S * k + 512 * sc:S * k + 512 * sc + 512],
                            start=(k == 0), stop=(k == 3))
                    nc.vector.tensor_copy(KTp[m][:, 512 * sc:512 * sc + 512], ps[:])

                def attn_v_unit(p, j, pt, av1, av2):
                    # both heads of pair p against j's V stripes; pt holds
                    # [h1 | h2] score-exp columns for this j
                    h1, h2 = 2 * p, 2 * p + 1
                    nc.tensor.matmul(
                        av1[:], lhsT=V_sb[:, VW * j + 65 * h1:VW * j + 65 * h1 + 65],
                        rhs=pt[:, 0:512],
                        start=(j == 0), stop=(j == JT - 1))
                    nc.tensor.matmul(
                        av2[:], lhsT=V_sb[:, VW * j + 65 * h2:VW * j + 65 * h2 + 65],
                        rhs=pt[:, 512:1024],
                        start=(j == 0), stop=(j == JT - 1))

                def finalize_head(h, av):
                    hp, hl = h // 2, h % 2
                    avc = rcp.tile([65, 512], F32, tag="avc", name=f"avc{h}")
                    rtmp = rcp.tile([1, 512], F32, tag="rt", name=f"rt{h}")
                    # denominator chain first - it's the critical path
                    # (broadcast + recip) - the big copy overlaps it
                    nc.vector.tensor_copy(rtmp[:], av[64:65, :])
                    rbc = rcp.tile([64, 512], F32, tag="rb", name=f"rb{h}")
                    rb2 = rcp.tile([64, 512], F32, tag="rb2", name=f"rb2{h}")
                    nc.gpsimd.partition_broadcast(rbc[:], rtmp[:])
                    nc.vector.tensor_copy(avc[:], av[:])
                    nc.vector.reciprocal_approx_fast(out=rb2[:], in_=rbc[:])
                    nc.vector.tensor_mul(
                        attT_sb[64 * hl:64 * hl + 64, 512 * hp:512 * hp + 512],
                        avc[0:64, :], rb2[:])

                def attention_pair(p, hooks, hold=0, lag=2):
                    """hooks: dict chunk->list of thunks run after that chunk's exp.
                    hold: delay attn@V issue until this chunk (issue-order guard
                    when the previous pair's deferred tail runs in our hooks).
                    lag: chunks between a unit's exp and its attn@V (deeper in
                    pair 0 so the V-proj drips ride the vst DMA curve)."""
                    h1, h2 = 2 * p, 2 * p + 1
                    av1 = psav.tile([65, 512], F32, tag="av1", name=f"av{h1}")
                    av2 = psav.tile([65, 512], F32, tag="av2", name=f"av{h2}")
                    q1 = qTz_sb[0:64, 512 * h1:512 * h1 + 512]
                    q2 = qTz_sb[64:128, 512 * h2:512 * h2 + 512]
                    backlog = []
                    pts = {}
                    for j in range(JT):
                        ps = pssc.tile([128, 1024], F32, tag="sc", name=f"sc{p}_{j}")
                        pt = ptp.tile([128, 1024], BF16, tag="pt", name=f"pt{p}_{j}")
                        nc.tensor.matmul(
                            ps[:, 0:512], lhsT=KTp[p][0:64, 128 * j:128 * j + 128],
                            rhs=q1, start=True, stop=True, tile_position=(0, 0))
                        nc.tensor.matmul(
                            ps[:, 512:1024], lhsT=KTp[p][64:128, 128 * j:128 * j + 128],
                            rhs=q2, start=True, stop=True, tile_position=(64, 0))
                        nc.scalar.activation(pt[:], ps[:], EXP, scale=0.125)
                        pts[j] = pt
                        for fn in hooks.get(j, ()):
                            fn()
                        if j >= lag and not (p == 0 and j - lag >= DEFER):
                            backlog.append(j - lag)
                        if j >= hold:
                            for bj in backlog:
                                attn_v_unit(p, bj, pts[bj], av1, av2)
                            backlog.clear()
                    for bj in backlog:
                        attn_v_unit(p, bj, pts[bj], av1, av2)
                    for bj in range(JT - lag, JT):
                        if not (p == 0 and bj >= DEFER):
                            attn_v_unit(p, bj, pts[bj], av1, av2)
                    return av1, av2, pts

                # ---- prolog: stage everything; q proj; K0 sc0 ----
                Wq_sb = stp.tile([128, 2048], BF16, tag="wq")
                xqT_sb = stp.tile([128, 2048], BF16, tag="xq")
                kst = stp.tile([128, 4 * S], BF16, tag="ks", name="kstage")
                vst = stp.tile([128, 4 * S], BF16, tag="vs", name="vstage")
                # per-k-chunk loads let the first q_proj matmuls start as
                # soon as ~256KB has landed instead of waiting for 1MB
                for k in range(4):
                    nc.sync.dma_start(
                        Wq_sb[:].rearrange("p (k n) -> p k n", k=4)[:, k, :],
                        Wq.ap().rearrange("(k p) n -> p k n", p=128)[:, k, :])
                    nc.sync.dma_start(
                        xqT_sb[:].rearrange("p (k s) -> p k s", k=4)[:, k, :],
                        xqT.ap().rearrange("(k p) s -> p k s", p=128)[:, k, :])
                nc.sync.dma_start(
                    Wk_sb[:].rearrange("p (k n) -> p k n", k=4),
                    Wk.ap().rearrange("(k p) n -> p k n", p=128))
                # interleave K / V staging by consumption deadline (the DMA
                # rail is the prolog+pair-0 limiter: ~10MB must land while
                # pair 0 runs). kst's first two 512-col chunks lead so the
                # first score chunks start ~8us earlier.
                staged_load(kst, keysT, nchunks=8, chunks=[0, 1])
                nc.sync.dma_start(
                    Wv_sb[:].rearrange("p (k n) -> p k n", k=4),
                    Wv.ap().rearrange("(k p) n -> p k n", p=128))
                staged_load(vst, valsT, nchunks=4, chunks=[0])
                for ci in range(1, 4):
                    staged_load(kst, keysT, nchunks=4, chunks=[ci])
                    staged_load(vst, valsT, nchunks=4, chunks=[ci])
                # Wo/bo aren't needed until fc (~200us): issue their DMAs from
                # the DVE queue after pair-1's copies so they don't steal
                # bandwidth from the attention-critical loads above
                def late_loads():
                    nc.gpsimd.dma_start(
                        Wo_sb[:].rearrange("p (k n) -> p k n", k=4),
                        Wo.ap().rearrange("(k p) n -> p k n", p=128))
                    nc.gpsimd.dma_start(bo_sb[:],
                                        bo.ap().rearrange("(m p) -> p m", p=128))

                q_proj()
                k_proj_group(0, 0)
                v_proj_group(0)
                v_proj_group(1)

                # ---- pair 0: V proj + K0 dripped JIT; K1 in the tail
                #      (pair 0 is DMA-stretched, so it has the PE slack) ----
                hooks0 = {}
                for s in range(1, 8):           # K0 sc s by chunk 4s
                    hooks0.setdefault(4 * s - 2, []).append(
                        lambda s=s: k_proj_group(0, s))
                for j in range(2, DEFER):       # V[j] by chunk j+1 (attnV lag 2)
                    hooks0.setdefault(j, []).append(
                        lambda j=j: v_proj_group(j))
                for s in range(4):              # K1 sc0-3 in pair-0's tail
                    hooks0.setdefault(26 + s, []).append(
                        lambda s=s: k_proj_group(1, s))
                av1_0, av2_0, pts0 = attention_pair(0, hooks0)

                # ---- pairs 1-3; pair-0 tail (V proj + attnV + finalize)
                #      runs inside pair 1's first hooks ----
                def make_hooks(p):
                    hooks = {}
                    if p == 1:
                        def tail(j):
                            def fn():
                                v_proj_group(j)
                                attn_v_unit(0, j, pts0[j], av1_0, av2_0)
                                if j == JT - 1:
                                    finalize_head(0, av1_0)
                                    finalize_head(1, av2_0)
                                    late_loads()
                            return fn
                        spread = [0, 1, 2, 4, 5, 6, 8, 9]
                        for i, j in enumerate(range(DEFER, JT)):
                            hooks.setdefault(spread[i], []).append(tail(j))
                    # own K tail JIT (sc s needed by chunk 4s); next pair's
                    # leading column-groups prime in our later hooks
                    if p == 1:
                        for s in range(4, 8):
                            hooks.setdefault(4 * s - 3, []).append(
                                lambda s=s: k_proj_group(p, s))
                        for i in range(2):
                            hooks.setdefault(26 + i, []).append(
                                lambda s=i: k_proj_group(2, s))
                    else:
                        for s in range(2, 8):
                            hooks.setdefault(4 * s - 3, []).append(
                                lambda s=s: k_proj_group(p, s))
                        if p < 3:
                            for i in range(2):
                                hooks.setdefault(26 + i, []).append(
                                    lambda s=i: k_proj_group(p + 1, s))
                    return hooks

                pts3 = None
                for p in (1, 2, 3):
                    av1, av2, ptsp = attention_pair(p, make_hooks(p),
                                                   hold=(11 if p == 1 else 0))
                    finalize_head(2 * p, av1)
                    finalize_head(2 * p + 1, av2)
                    pts3 = ptsp

                # keep PE busy across the finalize->fc dependency gap (~5us):
                # a >3.4us PE idle here re-throttles the HAM clock to 1.2GHz
                # and the whole fc phase then runs at half rate
                # rhs reads pair-3's late pt tiles so the static scheduler
                # cannot hoist these ahead of the tail of the attention phase
                warmf = pskp.tile([128, 512], F32, tag="kp", name="warmf")
                for i in range(18):
                    src = pts3[28 + (i % 4)]
                    nc.tensor.matmul(warmf[:], lhsT=ident[:],
                                     rhs=src[:, 0:512],
                                     start=True, stop=True)

            # ---- fc_out twice (fp32), then transpose to natural layout ----
            with tc.tile_pool(name="fc", bufs=3) as fcp, \
                 tc.tile_pool(name="ps_fc", bufs=2, space="PSUM") as psfc, \
                 tc.tile_pool(name="ps_tr", bufs=2, space="PSUM") as pstr:
                o1T = fcp.tile([128, 2048], BF16, tag="fcb", name="o1T")
                o2T = fcp.tile([128, 2048], BF16, tag="fcb", name="o2T")
                onat = fcp.tile([128, 2048], BF16, tag="fcb", name="onat")
                yv = y.ap().rearrange("(m p) f -> p m f", m=4, p=128)

                def fc_block(fsrc, fdst, m):
                    ps = psfc.tile([128, 512], F32, tag="fc")
                    for k in range(4):
                        nc.tensor.matmul(
                            ps[:], lhsT=Wo_sb[:, 512 * k + 128 * m:512 * k + 128 * m + 128],
                            rhs=fsrc[:, 512 * k:512 * k + 512],
                            start=(k == 0), stop=(k == 3))
                    nc.vector.tensor_scalar_add(
                        fdst[:, 512 * m:512 * m + 512], ps[:], bo_sb[:, m:m + 1])

                for m in range(4):
                    fc_block(attT_sb, o1T, m)
                for m in range(4):
                    fc_block(o1T, o2T, m)
                    # transposes of fc2 block m can start right away
                    # (kf-major): they hide under the next fc_block
                    for mm in range(4):
                        pst = pstr.tile([128, 128], BF16, tag="tr")
                        nc.tensor.transpose(
                            pst[:], o2T[:, 512 * m + 128 * mm:512 * m + 128 * mm + 128],
                            ident[:])
                        nc.vector.tensor_copy(
                            onat[:, 512 * mm + 128 * m:512 * mm + 128 * m + 128],
                            pst[:])
                    if m == 3:
                        for mm in range(4):
                            nc.sync.dma_start(
                                yv[:, mm, :],
                                onat[:, 512 * mm:512 * mm + 512])

    nc.compile()
    return nc


@functools.lru_cache(maxsize=1)
def _get_program():
    return _build_program()


def _make_in_maps(queries, keys, values, Wq, Wk, Wv, Wo, bo):
    q = np.asarray(queries, np.float32).reshape(S, D)
    kT = np.ascontiguousarray(np.asarray(keys, np.float32).reshape(S, D).T
                              ).astype(ml_dtypes.bfloat16)
    vT = np.ascontiguousarray(np.asarray(values, np.float32).reshape(S, D).T
                              ).astype(ml_dtypes.bfloat16)
    Wq = np.ascontiguousarray(np.asarray(Wq, np.float32)).astype(ml_dtypes.bfloat16)
    Wk = np.ascontiguousarray(np.asarray(Wk, np.float32)).astype(ml_dtypes.bfloat16)
    Wv = np.ascontiguousarray(np.asarray(Wv, np.float32)).astype(ml_dtypes.bfloat16)
    Wo = np.ascontiguousarray(np.asarray(Wo, np.float32)).astype(ml_dtypes.bfloat16)
    bo = np.ascontiguousarray(np.asarray(bo, np.float32))
    in_maps = []
    for c in range(NCORES):
        in_maps.append({
            "xqT": np.ascontiguousarray(q[c * CH:(c + 1) * CH].T).astype(ml_dtypes.bfloat16),
            "keysT": kT, "valsT": vT,
            "Wq": Wq, "Wk": Wk, "Wv": Wv, "Wo": Wo, "bo": bo,
        })
    return in_maps


def _run(in_maps, **kw):
    nc = _get_program()
    return run_bass_kernel_spmd(nc, in_maps, core_ids=list(range(NCORES)), **kw)


def kernel(queries, keys, values, Wq, Wk, Wv, Wo, bo):
    res = _run(_make_in_maps(queries, keys, values, Wq, Wk, Wv, Wo, bo))
    out = np.concatenate([res.results[c]["y"] for c in range(NCORES)],
                         axis=0).astype(np.float32)
    return out.reshape(1, S, D)


def run_traced(queries, keys, values, Wq, Wk, Wv, Wo, bo):
    """Like kernel() but with NTFF profiling; returns (output, BassKernelResults)."""
    import types
    import trn_agent_boot.trn_boot as _tb
    from concourse import bass_utils
    hook = _tb._ntff_profile_via_ctypes("/opt/axon/libaxon_pjrt.so")
    mod = types.ModuleType("antenv.axon_hooks")
    mod.get_axon_ntff_profile_hook = lambda: hook
    sys.modules["antenv.axon_hooks"] = mod
    bass_utils.upload_artifacts = lambda tmpdir: tmpdir
    res = _run(_make_in_maps(queries, keys, values, Wq, Wk, Wv, Wo, bo), trace=True)
    out = np.concatenate([res.results[c]["y"] for c in range(NCORES)],
                         axis=0).astype(np.float32)
    return out.reshape(1, S, D), res


# revision 40
# speedup vs baseline: 1.0047x; 1.0026x over previous
"""Trainium2 Bass kernel: MultiHeadSelfAttention (B=1, S=4096, D=512, H=8, DK=DV=64)
with fc_out applied twice.

Sharding: sequence-sharded across 8 cores (512 queries per core). Every core
receives the FULL keys/values (pre-transposed, bf16) and redundantly computes
the full K^T / V projections on-device (an AllGather measured ~125us wall for
1MB on this fabric - dead); attention + the two output projections run on the
core's own 512-query chunk. Host concatenates the 8 output chunks.

Layout notes:
  - heads are processed in PAIRS, lockstep over j-tiles. The scores^T tiles
    [seq_k(128) x seq_q(512)] for BOTH heads of a pair come out of PE in one
    ~220ns window via two concurrent row-tiled K=64 matmuls (tile_position
    (0,0)/(64,0)): head 2p streams its q through lanes 0-63 while head 2p+1
    streams through lanes 64-127. This fills the rhs xbus completely - 2x the
    throughput of the old zero-padded K=128 formulation.
  - KTp packs head pairs (head 2p rows 0-63, 2p+1 rows 64-127); qTz puts even
    heads on rows 0-63 and odd heads on rows 64-127 to match.
  - softmax denominator via a ones-column appended to each head's V (stride
    65): attn@V gives [65, 512] per head = output^T rows + exp-sum row. attn@V
    streams pt through all 128 lanes already (K=128) - irreducible, unchanged.
  - raw K^T and V^T stay RESIDENT in SBUF (4MB each; no reload churn), so
    projection drips can run any time: V + K0 + K1's head drip inside pair 0
    (V tail j>=DEFER deferred into pair 1's chunk hooks via parked pt tiles),
    K pair p's remaining column-groups JIT inside pair p itself.
  - PSUM: score chunks [128,1024] x2 bufs (4 banks) + av x2 + kproj x2 = 8.
  - input DMAs are deadline-ordered (the rail moves ~10MB while pair 0 runs);
    Wo/bo issue from the gpsimd queue mid-attention so they don't steal early
    bandwidth. Output y is bf16 (host casts back to f32).
  - the chip power-caps: >~3.4us PE idle halves the clock (HAM), but packing
    the pipeline perfectly trips the P0 power state (-20%% on ALL clocks).
    The pinned keep-warm matmuls before fc bridge the finalize dependency gap
    just enough to keep HAM at 8/8 without crossing the power budget.
"""
import sys, functools
sys.path.insert(0, "/opt/trn_rl_repo")
if "/root/.axon_site" not in sys.path:
    sys.path.insert(0, "/root/.axon_site")
import numpy as np
import ml_dtypes

import concourse.bass as bass
import concourse.tile as tile
from concourse import bacc, mybir, masks
from concourse.bass_utils import run_bass_kernel_spmd

NCORES = 8
S, D, H, DK = 4096, 512, 8, 64
CH = S // NCORES            # 512 sequence rows per core
VW = H * (DK + 1)           # 520: v row width incl. ones columns
JT = S // 128               # 32 seq_k tiles
DEFER = 24                  # pair-0 j-tiles >= DEFER run inside pair 1

F32 = mybir.dt.float32
BF16 = mybir.dt.bfloat16
EXP = mybir.ActivationFunctionType.Exp


def _build_program():
    nc = bacc.Bacc("TRN2", target_bir_lowering=False, debug=False,
                   num_devices=NCORES)

    xqT = nc.dram_tensor("xqT", [D, CH], BF16, kind="ExternalInput")
    keysT = nc.dram_tensor("keysT", [D, S], BF16, kind="ExternalInput")
    valsT = nc.dram_tensor("valsT", [D, S], BF16, kind="ExternalInput")
    Wq = nc.dram_tensor("Wq", [D, D], BF16, kind="ExternalInput")
    Wk = nc.dram_tensor("Wk", [D, D], BF16, kind="ExternalInput")
    Wv = nc.dram_tensor("Wv", [D, D], BF16, kind="ExternalInput")
    Wo = nc.dram_tensor("Wo", [D, D], BF16, kind="ExternalInput")
    bo = nc.dram_tensor("bo", [D], F32, kind="ExternalInput")
    y = nc.dram_tensor("y", [CH, D], BF16, kind="ExternalOutput")

    with tile.TileContext(nc) as tc:
        with tc.tile_pool(name="persist", bufs=1) as pp, \
             tc.tile_pool(name="kv", bufs=1) as kvp:

            Wo_sb = pp.tile([128, 2048], BF16, tag="wo")
            Wk_sb = pp.tile([128, 2048], BF16, tag="wk")
            Wv_sb = pp.tile([128, 2048], BF16, tag="wv")
            bo_sb = pp.tile([128, 4], F32, tag="bo")
            ident = pp.tile([128, 128], BF16, tag="id")
            qTz_sb = pp.tile([128, H * 512], BF16, tag="qt")
            attT_sb = pp.tile([128, 2048], BF16, tag="att")
            KTp = [kvp.tile([128, S], BF16, tag=f"kt{p}", name=f"KT{p}")
                   for p in range(H // 2)]
            # V natural [seq, head-stripes of 65 (64 + ones col)]
            V_sb = kvp.tile([128, JT * VW], BF16, tag="v")

            masks.make_identity(nc, ident[:])
            nc.gpsimd.memset(
                V_sb[:].rearrange("p (j h x) -> p j h x", j=JT, h=H, x=DK + 1)
                [:, :, :, DK:DK + 1], 1.0)

            with tc.tile_pool(name="stage", bufs=1) as stp, \
                 tc.tile_pool(name="pt", bufs=13) as ptp, \
                 tc.tile_pool(name="rc", bufs=2) as rcp, \
                 tc.tile_pool(name="ps_sc", bufs=2, space="PSUM") as pssc, \
                 tc.tile_pool(name="ps_kp", bufs=2, space="PSUM") as pskp, \
                 tc.tile_pool(name="ps_av", bufs=1, space="PSUM") as psav:

                def staged_load(dst_sb, src_dram, nchunks=8, chunks=None):
                    w = S // nchunks
                    dst = dst_sb[:].rearrange("p (k s) -> p k s", k=4)
                    srcv = src_dram.ap().rearrange("(k p) s -> p k s", p=128)
                    for ci in (range(nchunks) if chunks is None else chunks):
                        nc.sync.dma_start(dst[:, :, w * ci:w * ci + w],
                                          srcv[:, :, w * ci:w * ci + w])

                def q_proj():
                    for m in range(4):
                        ps = pskp.tile([128, 512], F32, tag="kp", name=f"qp{m}")
                        for k in range(4):
                            nc.tensor.matmul(
                                ps[:], lhsT=Wq_sb[:, 512 * k + 128 * m:512 * k + 128 * m + 128],
                                rhs=xqT_sb[:, 512 * k:512 * k + 512],
                                start=(k == 0), stop=(k == 3))
                        nc.vector.tensor_copy(
                            qTz_sb[0:64, 512 * (2 * m):512 * (2 * m) + 512], ps[0:64, :])
                        nc.vector.tensor_copy(
                            qTz_sb[64:128, 512 * (2 * m + 1):512 * (2 * m + 1) + 512],
                            ps[64:128, :])

                def v_proj_group(j):
                    ps = pskp.tile([128, 512], F32, tag="kp", name=f"vp{j}")
                    for k in range(4):
                        nc.tensor.matmul(
                            ps[:], lhsT=vst[:, S * k + 128 * j:S * k + 128 * j + 128],
                            rhs=Wv_sb[:, 512 * k:512 * k + 512],
                            start=(k == 0), stop=(k == 3))
                    dst = V_sb[:, VW * j:VW * j + VW].rearrange(
                        "p (h x) -> p h x", h=H, x=DK + 1)[:, :, 0:DK]
                    nc.vector.tensor_copy(
                        dst, ps[:].rearrange("p (h x) -> p h x", h=H, x=DK))

                def k_proj_group(m, sc):
                    ps = pskp.tile([128, 512], F32, tag="kp", name=f"kp{m}_{sc}")
                    for k in range(4):
                        nc.tensor.matmul(
                            ps[:], lhsT=Wk_sb[:, 512 * k + 128 * m:512 * k + 128 * m + 128],
                            rhs=kst[:, S * k + 512 * sc:S * k + 512 * sc + 512],
                            start=(k == 0), stop=(k == 3))
                    nc.vector.tensor_copy(KTp[m][:, 512 * sc:512 * sc + 512], ps[:])

                def attn_v_unit(p, j, pt, av1, av2):
                    # both heads of pair p against j's V stripes; pt holds
                    # [h1 | h2] score-exp columns for this j
                    h1, h2 = 2 * p, 2 * p + 1
                    nc.tensor.matmul(
                        av1[:], lhsT=V_sb[:, VW * j + 65 * h1:VW * j + 65 * h1 + 65],
                        rhs=pt[:, 0:512],
                        start=(j == 0), stop=(j == JT - 1))
                    nc.tensor.matmul(
                        av2[:], lhsT=V_sb[:, VW * j + 65 * h2:VW * j + 65 * h2 + 65],
                        rhs=pt[:, 512:1024],
                        start=(j == 0), stop=(j == JT - 1))

                def finalize_head(h, av):
                    hp, hl = h // 2, h % 2
                    avc = rcp.tile([65, 512], F32, tag="avc", name=f"avc{h}")
                    rtmp = rcp.tile([1, 512], F32, tag="rt", name=f"rt{h}")
                    # denominator chain first - it's the critical path
                    # (broadcast + recip) - the big copy overlaps it
                    nc.vector.tensor_copy(rtmp[:], av[64:65, :])
                    rbc = rcp.tile([64, 512], F32, tag="rb", name=f"rb{h}")
                    rb2 = rcp.tile([64, 512], F32, tag="rb2", name=f"rb2{h}")
                    nc.gpsimd.partition_broadcast(rbc[:], rtmp[:])
                    nc.vector.tensor_copy(avc[:], av[:])
                    nc.vector.reciprocal_approx_fast(out=rb2[:], in_=rbc[:])
                    nc.vector.tensor_mul(
                        attT_sb[64 * hl:64 * hl + 64, 512 * hp:512 * hp + 512],
                        avc[0:64, :], rb2[:])

                def attention_pair(p, hooks, hold=0, lag=2):
                    """hooks: dict chunk->list of thunks run after that chunk's exp.
                    hold: delay attn@V issue until this chunk (issue-order guard
                    when the previous pair's deferred tail runs in our hooks).
                    lag: chunks between a unit's exp and its attn@V (deeper in
                    pair 0 so the V-proj drips ride the vst DMA curve)."""
                    h1, h2 = 2 * p, 2 * p + 1
                    av1 = psav.tile([65, 512], F32, tag="av1", name=f"av{h1}")
                    av2 = psav.tile([65, 512], F32, tag="av2", name=f"av{h2}")
                    q1 = qTz_sb[0:64, 512 * h1:512 * h1 + 512]
                    q2 = qTz_sb[64:128, 512 * h2:512 * h2 + 512]
                    backlog = []
                    pts = {}
                    for j in range(JT):
                        ps = pssc.tile([128, 1024], F32, tag="sc", name=f"sc{p}_{j}")
                        pt = ptp.tile([128, 1024], BF16, tag="pt", name=f"pt{p}_{j}")
                        nc.tensor.matmul(
                            ps[:, 0:512], lhsT=KTp[p][0:64, 128 * j:128 * j + 128],
                            rhs=q1, start=True, stop=True, tile_position=(0, 0))
                        nc.tensor.matmul(
                            ps[:, 512:1024], lhsT=KTp[p][64:128, 128 * j:128 * j + 128],
                            rhs=q2, start=True, stop=True, tile_position=(64, 0))
                        nc.scalar.activation(pt[:], ps[:], EXP, scale=0.125)
                        pts[j] = pt
                        for fn in hooks.get(j, ()):
                            fn()
                        if j >= lag and not (p == 0 and j - lag >= DEFER):
                            backlog.append(j - lag)
                        if j >= hold:
                            for bj in backlog:
                                attn_v_unit(p, bj, pts[bj], av1, av2)
                            backlog.clear()
                    for bj in backlog:
                        attn_v_unit(p, bj, pts[bj], av1, av2)
                    for bj in range(JT - lag, JT):
                        if not (p == 0 and bj >= DEFER):
                            attn_v_unit(p, bj, pts[bj], av1, av2)
                    return av1, av2, pts

                # ---- prolog: stage everything; q proj; K0 sc0 ----
                Wq_sb = stp.tile([128, 2048], BF16, tag="wq")
                xqT_sb = stp.tile([128, 2048], BF16, tag="xq")
                kst = stp.tile([128, 4 * S], BF16, tag="ks", name="kstage")
                vst = stp.tile([128, 4 * S], BF16, tag="vs", name="vstage")
                # per-k-chunk loads let the first q_proj matmuls start as
                # soon as ~256KB has landed instead of waiting for 1MB
                for k in range(4):
                    nc.sync.dma_start(
                        Wq_sb[:].rearrange("p (k n) -> p k n", k=4)[:, k, :],
                        Wq.ap().rearrange("(k p) n -> p k n", p=128)[:, k, :])
                    nc.sync.dma_start(
                        xqT_sb[:].rearrange("p (k s) -> p k s", k=4)[:, k, :],
                        xqT.ap().rearrange("(k p) s -> p k s", p=128)[:, k, :])
                nc.sync.dma_start(
                    Wk_sb[:].rearrange("p (k n) -> p k n", k=4),
                    Wk.ap().rearrange("(k p) n -> p k n", p=128))
                # interleave K / V staging by consumption deadline (the DMA
                # rail is the prolog+pair-0 limiter: ~10MB must land while
                # pair 0 runs). kst's first two 512-col chunks lead so the
                # first score chunks start ~8us earlier.
                staged_load(kst, keysT, nchunks=8, chunks=[0, 1])
                nc.sync.dma_start(
                    Wv_sb[:].rearrange("p (k n) -> p k n", k=4),
                    Wv.ap().rearrange("(k p) n -> p k n", p=128))
                staged_load(vst, valsT, nchunks=4, chunks=[0])
                for ci in range(1, 4):
                    staged_load(kst, keysT, nchunks=4, chunks=[ci])
                    staged_load(vst, valsT, nchunks=4, chunks=[ci])
                # Wo/bo aren't needed until fc (~200us): issue their DMAs from
                # the DVE queue after pair-1's copies so they don't steal
                # bandwidth from the attention-critical loads above
                def late_loads():
                    nc.gpsimd.dma_start(
                        Wo_sb[:].rearrange("p (k n) -> p k n", k=4),
                        Wo.ap().rearrange("(k p) n -> p k n", p=128))
                    nc.gpsimd.dma_start(bo_sb[:],
                                        bo.ap().rearrange("(m p) -> p m", p=128))

                q_proj()
                k_proj_group(0, 0)
                v_proj_group(0)
                v_proj_group(1)

                # ---- pair 0: V proj + K0 dripped JIT; K1 in the tail
                #      (pair 0 is DMA-stretched, so it has the PE slack) ----
                hooks0 = {}
                for s in range(1, 8):           # K0 sc s by chunk 4s
                    hooks0.setdefault(4 * s - 2, []).append(
                        lambda s=s: k_proj_group(0, s))
                for j in range(2, DEFER):       # V[j] by chunk j+1 (attnV lag 2)
                    hooks0.setdefault(j, []).append(
                        lambda j=j: v_proj_group(j))
                for s in range(4):              # K1 sc0-3 in pair-0's tail
                    hooks0.setdefault(26 + s, []).append(
                        lambda s=s: k_proj_group(1, s))
                av1_0, av2_0, pts0 = attention_pair(0, hooks0)

                # ---- pairs 1-3; pair-0 tail (V proj + attnV + finalize)
                #      runs inside pair 1's first hooks ----
                def make_hooks(p):
                    hooks = {}
                    if p == 1:
                        def tail(j):
                            def fn():
                                v_proj_group(j)
                                attn_v_unit(0, j, pts0[j], av1_0, av2_0)
                                if j == JT - 1:
                                    finalize_head(0, av1_0)
                                    finalize_head(1, av2_0)
                                    late_loads()
                            return fn
                        spread = [0, 1, 2, 4, 5, 6, 8, 9]
                        for i, j in enumerate(range(DEFER, JT)):
                            hooks.setdefault(spread[i], []).append(tail(j))
                    # own K tail JIT (sc s needed by chunk 4s); next pair's
                    # leading column-groups prime in our later hooks
                    if p == 1:
                        for s in range(4, 8):
                            hooks.setdefault(4 * s - 3, []).append(
                                lambda s=s: k_proj_group(p, s))
                        for i in range(2):
                            hooks.setdefault(26 + i, []).append(
                                lambda s=i: k_proj_group(2, s))
                    else:
                        for s in range(2, 8):
                            hooks.setdefault(4 * s - 3, []).append(
                                lambda s=s: k_proj_group(p, s))
                        if p < 3:
                            for i in range(2):
                                hooks.setdefault(26 + i, []).append(
                                    lambda s=i: k_proj_group(p + 1, s))
                    return hooks

                pts3 = None
                for p in (1, 2, 3):
                    av1, av2, ptsp = attention_pair(p, make_hooks(p),
                                                   hold=(11 if p == 1 else 0))
                    finalize_head(2 * p, av1)
                    finalize_head(2 * p + 1, av2)
                    pts3 = ptsp

                # keep PE busy across the finalize->fc dependency gap (~5us):
                # a >3.4us PE idle here re-throttles the HAM clock to 1.2GHz
                # and the whole fc phase then runs at half rate
                # rhs reads pair-3's late pt tiles so the static scheduler
                # cannot hoist these ahead of the tail of the attention phase
                warmf = pskp.tile([128, 512], F32, tag="kp", name="warmf")
                for i in range(18):
                    src = pts3[28 + (i % 4)]
                    nc.tensor.matmul(warmf[:], lhsT=ident[:],
                                     rhs=src[:, 0:512],
                                     start=True, stop=True)

            # ---- fc_out twice (fp32), then transpose to natural layout ----
            with tc.tile_pool(name="fc", bufs=3) as fcp, \
                 tc.tile_pool(name="ps_fc", bufs=2, space="PSUM") as psfc, \
                 tc.tile_pool(name="ps_tr", bufs=2, space="PSUM") as pstr:
                o1T = fcp.tile([128, 2048], BF16, tag="fcb", name="o1T")
                o2T = fcp.tile([128, 2048], BF16, tag="fcb", name="o2T")
                onat = fcp.tile([128, 2048], BF16, tag="fcb", name="onat")
                yv = y.ap().rearrange("(m p) f -> p m f", m=4, p=128)

                def fc_block(fsrc, fdst, m):
                    ps = psfc.tile([128, 512], F32, tag="fc")
                    for k in range(4):
                        nc.tensor.matmul(
                            ps[:], lhsT=Wo_sb[:, 512 * k + 128 * m:512 * k + 128 * m + 128],
                            rhs=fsrc[:, 512 * k:512 * k + 512],
                            start=(k == 0), stop=(k == 3))
                    nc.vector.tensor_scalar_add(
                        fdst[:, 512 * m:512 * m + 512], ps[:], bo_sb[:, m:m + 1])

                for m in range(4):
                    fc_block(attT_sb, o1T, m)
                for m in range(4):
                    fc_block(o1T, o2T, m)
                    # transposes of fc2 block m can start right away
                    # (kf-major): they hide under the next fc_block
                    for mm in range(4):
                        pst = pstr.tile([128, 128], BF16, tag="tr")
                        nc.tensor.transpose(
                            pst[:], o2T[:, 512 * m + 128 * mm:512 * m + 128 * mm + 128],
                            ident[:])
                        nc.vector.tensor_copy(
                            onat[:, 512 * mm + 128 * m:512 * mm + 128 * m + 128],
                            pst[:])
                    if m == 3:
                        for mm in range(4):
                            nc.sync.dma_start(
                                yv[:, mm, :],
                                onat[:, 512 * mm:512 * mm + 512])

    nc.compile()
    return nc


@functools.lru_cache(maxsize=1)
def _get_program():
    return _build_program()


def _make_in_maps(queries, keys, values, Wq, Wk, Wv, Wo, bo):
    q = np.asarray(queries, np.float32).reshape(S, D)
    kT = np.ascontiguousarray(np.asarray(keys, np.float32).reshape(S, D).T
                              ).astype(ml_dtypes.bfloat16)
    vT = np.ascontiguousarray(np.asarray(values, np.float32).reshape(S, D).T
                              ).astype(ml_dtypes.bfloat16)
    Wq = np.ascontiguousarray(np.asarray(Wq, np.float32)).astype(ml_dtypes.bfloat16)
    Wk = np.ascontiguousarray(np.asarray(Wk, np.float32)).astype(ml_dtypes.bfloat16)
    Wv = np.ascontiguousarray(np.asarray(Wv, np.float32)).astype(ml_dtypes.bfloat16)
    Wo = np.ascontiguousarray(np.asarray(Wo, np.float32)).astype(ml_dtypes.bfloat16)
    bo = np.ascontiguousarray(np.asarray(bo, np.float32))
    in_maps = []
    for c in range(NCORES):
        in_maps.append({
            "xqT": np.ascontiguousarray(q[c * CH:(c + 1) * CH].T).astype(ml_dtypes.bfloat16),
            "keysT": kT, "valsT": vT,
            "Wq": Wq, "Wk": Wk, "Wv": Wv, "Wo": Wo, "bo": bo,
        })
    return in_maps


def _run(in_maps, **kw):
    nc = _get_program()
    return run_bass_kernel_spmd(nc, in_maps, core_ids=list(range(NCORES)), **kw)


def kernel(queries, keys, values, Wq, Wk, Wv, Wo, bo):
    res = _run(_make_in_maps(queries, keys, values, Wq, Wk, Wv, Wo, bo))
    out = np.concatenate([res.results[c]["y"] for c in range(NCORES)],
                         axis=0).astype(np.float32)
    return out.reshape(1, S, D)


def run_traced(queries, keys, values, Wq, Wk, Wv, Wo, bo):
    """Like kernel() but with NTFF profiling; returns (output, BassKernelResults)."""
    import types
    import trn_agent_boot.trn_boot as _tb
    from concourse import bass_utils
    hook = _tb._ntff_profile_via_ctypes("/opt/axon/libaxon_pjrt.so")
    mod = types.ModuleType("antenv.axon_hooks")
    mod.get_axon_ntff_profile_hook = lambda: hook
    sys.modules["antenv.axon_hooks"] = mod
    bass_utils.upload_artifacts = lambda tmpdir: tmpdir
    res = _run(_make_in_maps(queries, keys, values, Wq, Wk, Wv, Wo, bo), trace=True)
    out = np.concatenate([res.results[c]["y"] for c in range(NCORES)],
                         axis=0).astype(np.float32)
    return out.reshape(1, S, D), res
